# revision 11
# baseline (speedup 1.0000x reference)
"""BiMambaV2 Trainium2 kernel (v2).

Sharding: 8 cores = 4 samples x 2 directions (SPMD, one program).
Each core computes a full mamba pass for one (sample, direction); the
host feeds the backward core a time-reversed sequence and flips its
output back before summing.

Engine plan (per core):
  PE     in_proj(x,z), depthwise causal conv as 4 diagonal-matmul PSUM
         taps, x_proj, dt_proj, y = sum_n C_n*h_n via identity-matmul
         PSUM accumulation (D*u folded in as a diag(D) tap), state-carry
         extraction (1-col matmuls), out_proj.
  ACT    silu(u) (stage A), softplus via Exp+Ln, the 512
         dA=exp(A_n*delta) tiles, PSUM->SBUF copies. silu(z) is one
         batch at the chunk boundary so the activation table only
         switches 4 times in the whole kernel.
  DVE    the 512 tensor_tensor_scan recurrences (bf16 in/out, fp32
         internal state), most dBu=delta*u*B_n muls, half the C*h muls,
         gating.
  POOL   overflow lane for the remaining dBu / C*h muls.
  SP/ACT DMA queues; nothing issued from DVE or PE.

Scan runs in 2 chunks of 1024 (the bf16 scan rate degrades at 2048);
the inter-chunk state is carried by 1-column identity matmuls into a
PSUM tile, copied once per d-tile to SBUF, and fed to chunk-1 scans via
the `initial=` AP. Chunk-0 gating is deferred to the boundary batch
(ungated y round-trips DRAM) so silu(z) never interleaves with the exp
table. Numerics verified at rel err ~5e-3 vs the fp32 reference.
"""

import numpy as np

D_MODEL = 1024
D_INNER = 2048
N_STATE = 16
DT_RANK = 64
BATCH = 4
SEQLEN = 2048
K_CONV = 4

P = 128
TC = 1024                    # scan chunk
NCH = SEQLEN // TC           # 2
DT_TILES = D_INNER // P      # 16
KM_TILES = D_MODEL // P      # 8
R = DT_RANK + 2 * N_STATE    # 96
NMM = SEQLEN // 512          # 4

# which ops run on the Pool engine (overflow lane for the DVE)
POOL_DBU = frozenset((10, 11, 12, 13, 14, 15))
POOL_TN = frozenset((8, 9, 10, 11, 12, 13, 14, 15))

_CACHE = {}
_LAST_IN_MAPS = None


def _build():
    import concourse.bass as bass
    import concourse.bacc as bacc
    import concourse.tile as tile
    from concourse import mybir
    from concourse.masks import make_identity

    f32 = mybir.dt.float32
    bf16 = mybir.dt.bfloat16
    AF = mybir.ActivationFunctionType
    OP = mybir.AluOpType

    nc = bacc.Bacc("TRN2", target_bir_lowering=False, debug=False, num_devices=8)

    # ---- per-core inputs ----
    hT = nc.dram_tensor("hT", [D_MODEL, SEQLEN], bf16, kind="ExternalInput")
    w_inT = nc.dram_tensor("w_inT", [D_MODEL, 2 * D_INNER], bf16, kind="ExternalInput")
    conv_w = nc.dram_tensor("conv_w", [D_INNER, K_CONV], f32, kind="ExternalInput")
    conv_b = nc.dram_tensor("conv_b", [D_INNER, 1], f32, kind="ExternalInput")
    x_projT = nc.dram_tensor("x_projT", [D_INNER, R], bf16, kind="ExternalInput")
    dt_projT = nc.dram_tensor("dt_projT", [DT_RANK, D_INNER], bf16, kind="ExternalInput")
    dt_b = nc.dram_tensor("dt_b", [D_INNER, 1], f32, kind="ExternalInput")
    A_m = nc.dram_tensor("A_m", [D_INNER, N_STATE], f32, kind="ExternalInput")
    D_v = nc.dram_tensor("D_v", [D_INNER, 1], f32, kind="ExternalInput")
    w_outT = nc.dram_tensor("w_outT", [D_INNER, D_MODEL], bf16, kind="ExternalInput")

    out = nc.dram_tensor("out", [SEQLEN, D_MODEL], f32, kind="ExternalOutput")

    # ---- DRAM intermediates (bf16) ----
    u_d = nc.dram_tensor("u_d", [D_INNER, SEQLEN], bf16)
    delta_d = nc.dram_tensor("delta_d", [D_INNER, SEQLEN], bf16)
    z_d = nc.dram_tensor("z_d", [D_INNER, SEQLEN], bf16)
    sz_d = nc.dram_tensor("sz_d", [D_INNER, TC], bf16)       # chunk-1 half only
    py_d = nc.dram_tensor("py_d", [D_INNER, TC], bf16)       # chunk-0 ungated y
    y_d = nc.dram_tensor("y_d", [D_INNER, SEQLEN], bf16)
    xdbl_d = nc.dram_tensor("xdbl_d", [R, SEQLEN], bf16)

    with tile.TileContext(nc) as tc:
        import contextlib
        stack = contextlib.ExitStack()
        const = stack.enter_context(tc.tile_pool(name="const", bufs=1))

        ident = const.tile([P, P], bf16, tag="ident")
        make_identity(nc, ident)

        dtp_sb = const.tile([DT_RANK, DT_TILES, P], bf16, tag="dtp")
        dsrc = bass.AP(tensor=dt_projT.ap().tensor, offset=0,
                       ap=[[D_INNER, DT_RANK], [P, DT_TILES], [1, P]])
        nc.sync.dma_start(out=dtp_sb[:], in_=dsrc)
        xp_sb = const.tile([P, DT_TILES, R], bf16, tag="xp")
        xsrc = bass.AP(tensor=x_projT.ap().tensor, offset=0,
                       ap=[[R, P], [P * R, DT_TILES], [1, R]])
        nc.sync.dma_start(out=xp_sb[:], in_=xsrc)

        a_sb, cw_sb, cb_sb, dtb_sb, dD_sb, hl_sb = [], [], [], [], [], []
        for dt in range(DT_TILES):
            a = const.tile([P, N_STATE], f32, tag=f"a{dt}")
            nc.sync.dma_start(out=a[:], in_=A_m[dt * P:(dt + 1) * P, :])
            a_sb.append(a)
            cw = const.tile([P, K_CONV], f32, tag=f"cw{dt}")
            nc.sync.dma_start(out=cw[:], in_=conv_w[dt * P:(dt + 1) * P, :])
            cw_sb.append(cw)
            cb = const.tile([P, 1], f32, tag=f"cb{dt}")
            nc.sync.dma_start(out=cb[:], in_=conv_b[dt * P:(dt + 1) * P, :])
            cb_sb.append(cb)
            db = const.tile([P, 1], f32, tag=f"db{dt}")
            nc.sync.dma_start(out=db[:], in_=dt_b[dt * P:(dt + 1) * P, :])
            dtb_sb.append(db)
            dv = const.tile([P, 1], f32, tag=f"dv{dt}")
            nc.sync.dma_start(out=dv[:], in_=D_v[dt * P:(dt + 1) * P, :])
            dD = const.tile([P, P], bf16, tag=f"dD{dt}")
            nc.vector.scalar_tensor_tensor(out=dD[:], in0=ident[:],
                                           scalar=dv[:, 0:1], in1=ident[:],
                                           op0=OP.mult, op1=OP.bypass)
            dD_sb.append(dD)
            hl = const.tile([P, N_STATE], f32, tag=f"hl{dt}")
            hl_sb.append(hl)

        xdbl_sb = const.tile([R, SEQLEN], bf16, tag="xdbl_sb")

        # ht stays alive through stage A and chunk 0 (z rows read it)
        ht_pool = stack.enter_context(tc.tile_pool(name="htp", bufs=1))
        ht_sb = ht_pool.tile([P, KM_TILES, SEQLEN], bf16, tag="ht")
        for k in range(KM_TILES):
            hsrc = bass.AP(tensor=hT.ap().tensor, offset=k * P * SEQLEN,
                           ap=[[SEQLEN, P], [1, SEQLEN]])
            nc.sync.dma_start(out=ht_sb[:, k, :], in_=hsrc)

        # ---------------- stage A ----------------
        # x_proj PSUM accumulators live across the whole m loop so the
        # contraction over d_inner proceeds as each u tile is produced.
        with tc.tile_pool(name="s3px", bufs=1, space="PSUM") as s3px:
            psx_t = []
            for nn in range(NMM):
                pxt = s3px.tile([R, 512], f32, tag=f"psx{nn}")
                psx_t.append(pxt)
            with tc.tile_pool(name="s1w", bufs=3) as s1w, \
                 tc.tile_pool(name="s1x", bufs=2) as s1x, \
                 tc.tile_pool(name="s1u", bufs=2) as s1u, \
                 tc.tile_pool(name="s1p", bufs=2, space="PSUM") as s1p, \
                 tc.tile_pool(name="s1pc", bufs=2, space="PSUM") as s1pc:
                # x rows: in_proj -> conv (PE diag taps) -> silu -> u_d
                for m in range(DT_TILES):
                    wt = s1w.tile([P, KM_TILES, P], bf16, tag="wt")
                    wsrc = bass.AP(tensor=w_inT.ap().tensor, offset=m * P,
                                   ap=[[2 * D_INNER, P], [P * 2 * D_INNER, KM_TILES], [1, P]])
                    nc.sync.dma_start(out=wt[:], in_=wsrc)
                    dcw = s1w.tile([P, K_CONV, P], bf16, tag="dcw")
                    for k in range(K_CONV):
                        nc.vector.scalar_tensor_tensor(out=dcw[:, k, :], in0=ident[:],
                                                       scalar=cw_sb[m][:, k:k + 1],
                                                       in1=ident[:],
                                                       op0=OP.mult, op1=OP.bypass)
                    xs = s1x.tile([P, K_CONV - 1 + SEQLEN], bf16, tag="xs")
                    nc.vector.memset(xs[:, 0:K_CONV - 1], 0.0)
                    for j in range(NMM):
                        ps = s1p.tile([P, 512], f32, tag="ps")
                        for k in range(KM_TILES):
                            nc.tensor.matmul(ps[:], wt[:, k, :],
                                             ht_sb[:, k, j * 512:(j + 1) * 512],
                                             start=(k == 0), stop=(k == KM_TILES - 1))
                        nc.scalar.copy(out=xs[:, K_CONV - 1 + j * 512:K_CONV - 1 + (j + 1) * 512],
                                       in_=ps[:])
                    ut = s1u.tile([P, SEQLEN], bf16, tag="ut")
                    for j in range(NMM):
                        psc = s1pc.tile([P, 512], f32, tag="psc")
                        for k in range(K_CONV):
                            # tap k reads x[t - (K-1-k)] -> xs offset j*512 + k
                            nc.tensor.matmul(psc[:], dcw[:, k, :],
                                             xs[:, j * 512 + k:j * 512 + k + 512],
                                             start=(k == 0), stop=(k == K_CONV - 1))
                        nc.scalar.activation(out=ut[:, j * 512:(j + 1) * 512], in_=psc[:],
                                             func=AF.Silu, bias=cb_sb[m][:, 0:1], scale=1.0)
                    nc.sync.dma_start(out=u_d[m * P:(m + 1) * P, :], in_=ut[:])
                    # incremental x_proj taps for this u tile
                    for nn in range(NMM):
                        nc.tensor.matmul(psx_t[nn][:], xp_sb[:, m, :],
                                         ut[:, nn * 512:(nn + 1) * 512],
                                         start=(m == 0), stop=(m == DT_TILES - 1))

            for nn in range(NMM):
                nc.scalar.copy(out=xdbl_sb[:, nn * 512:(nn + 1) * 512], in_=psx_t[nn][:])
                nc.sync.dma_start(out=xdbl_d[:, nn * 512:(nn + 1) * 512],
                                  in_=xdbl_sb[:, nn * 512:(nn + 1) * 512])

            # dt_proj + softplus (Exp then Ln, natural_log_exp table)
            with tc.tile_pool(name="s4e", bufs=2) as s4e, \
                 tc.tile_pool(name="s4d", bufs=2) as s4d, \
                 tc.tile_pool(name="s4p", bufs=2, space="PSUM") as s4p:
                for m in range(DT_TILES):
                    dm = s4d.tile([P, SEQLEN], bf16, tag="dm")
                    for nn in range(NMM):
                        ps4 = s4p.tile([P, 512], f32, tag="ps4")
                        nc.tensor.matmul(ps4[:], dtp_sb[:, m, :],
                                         xdbl_sb[0:DT_RANK, nn * 512:(nn + 1) * 512],
                                         start=True, stop=True)
                        ee = s4e.tile([P, 512], f32, tag="ee")
                        nc.scalar.activation(out=ee[:], in_=ps4[:], func=AF.Exp,
                                             bias=dtb_sb[m][:, 0:1], scale=1.0)
                        nc.scalar.activation(out=dm[:, nn * 512:(nn + 1) * 512],
                                             in_=ee[:], func=AF.Ln, bias=1.0, scale=1.0)
                    nc.sync.dma_start(out=delta_d[m * P:(m + 1) * P, :], in_=dm[:])

        # ---------------- scan phase ----------------
        # broadcast tiles, rewritten per chunk
        bc_pool = stack.enter_context(tc.tile_pool(name="bc", bufs=1))
        b_sb, c_sb = [], []
        for n in range(N_STATE):
            bt = bc_pool.tile([P, TC], bf16, tag=f"bB{n}")
            b_sb.append(bt)
            ct = bc_pool.tile([P, TC], bf16, tag=f"bC{n}")
            c_sb.append(ct)

        with tc.tile_pool(name="s5s", bufs=2) as s5s, \
             tc.tile_pool(name="s5a", bufs=3) as s5a, \
             tc.tile_pool(name="s5b", bufs=4) as s5bp, \
             tc.tile_pool(name="s5h", bufs=6) as s5h, \
             tc.tile_pool(name="s5t", bufs=3) as s5t, \
             tc.tile_pool(name="s5y", bufs=2) as s5y, \
             tc.tile_pool(name="s5p", bufs=2, space="PSUM") as s5p, \
             tc.tile_pool(name="s5hp", bufs=2, space="PSUM") as s5hp:

            def scan_tile(c, dt):
                cs = c * TC
                dl = s5s.tile([P, TC], bf16, tag="dl")
                nc.sync.dma_start(out=dl[:], in_=delta_d[dt * P:(dt + 1) * P, cs:cs + TC])
                ut = s5s.tile([P, TC], bf16, tag="ut")
                nc.sync.dma_start(out=ut[:], in_=u_d[dt * P:(dt + 1) * P, cs:cs + TC])
                dlu = s5s.tile([P, TC], bf16, tag="dlu")
                nc.vector.tensor_mul(out=dlu[:], in0=dl[:], in1=ut[:])
                dlu2 = s5s.tile([P, TC], bf16, tag="dlu2")
                nc.scalar.copy(out=dlu2[:], in_=dlu[:])

                psy = s5p.tile([P, TC], f32, tag="psy")
                if c == 0:
                    hlp = s5hp.tile([P, N_STATE], f32, tag="hlp")
                else:
                    hlp = None

                for n in range(N_STATE):
                    dA = s5a.tile([P, TC], bf16, tag="dA")
                    nc.scalar.activation(out=dA[:], in_=dl[:], func=AF.Exp,
                                         scale=a_sb[dt][:, n:n + 1])
                    dBu = s5bp.tile([P, TC], bf16, tag="dBu")
                    if n in POOL_DBU:
                        nc.gpsimd.tensor_mul(out=dBu[:], in0=dlu2[:], in1=b_sb[n][:])
                    else:
                        nc.vector.tensor_mul(out=dBu[:], in0=dlu[:], in1=b_sb[n][:])
                    hn = s5h.tile([P, TC], bf16, tag="hn")
                    init = 0.0 if c == 0 else hl_sb[dt][:, n:n + 1]
                    nc.vector.tensor_tensor_scan(out=hn[:], data0=dA[:], data1=dBu[:],
                                                 initial=init, op0=OP.mult, op1=OP.add)
                    if c == 0:
                        nc.tensor.matmul(hlp[:, n:n + 1], ident[:], hn[:, TC - 1:TC],
                                         start=True, stop=True)
                    tn = s5t.tile([P, TC], bf16, tag="tn")
                    teng = nc.gpsimd if n in POOL_TN else nc.vector
                    teng.tensor_mul(out=tn[:], in0=hn[:], in1=c_sb[n][:])
                    for hh in range(TC // 512):
                        nc.tensor.matmul(psy[:, hh * 512:(hh + 1) * 512], ident[:],
                                         tn[:, hh * 512:(hh + 1) * 512],
                                         start=(n == 0), stop=False)
                # D*u tap via diag(D), closes the accumulation
                for hh in range(TC // 512):
                    nc.tensor.matmul(psy[:, hh * 512:(hh + 1) * 512], dD_sb[dt][:],
                                     ut[:, hh * 512:(hh + 1) * 512],
                                     start=False, stop=True)
                if c == 0:
                    nc.scalar.copy(out=hl_sb[dt][:], in_=hlp[:])
                py = s5y.tile([P, TC], bf16, tag="py")
                nc.scalar.copy(out=py[:], in_=psy[:])
                return py, ut

            # ---- chunk 0 (ht alive: z rows run on PE under the scans) ----
            for n in range(N_STATE):
                bsrc = bass.AP(tensor=xdbl_d.ap().tensor,
                               offset=(DT_RANK + n) * SEQLEN,
                               ap=[[0, P], [1, TC]])
                nc.sync.dma_start(out=b_sb[n][:], in_=bsrc)
                csrc = bass.AP(tensor=xdbl_d.ap().tensor,
                               offset=(DT_RANK + N_STATE + n) * SEQLEN,
                               ap=[[0, P], [1, TC]])
                nc.sync.dma_start(out=c_sb[n][:], in_=csrc)

            with tc.tile_pool(name="zw", bufs=2) as zw, \
                 tc.tile_pool(name="zc", bufs=3) as zcp, \
                 tc.tile_pool(name="zp", bufs=2, space="PSUM") as zp:
                for dt in range(DT_TILES):
                    py, _ = scan_tile(0, dt)
                    nc.sync.dma_start(out=py_d[dt * P:(dt + 1) * P, :], in_=py[:])
                    # z half of in_proj on PE under the scans; raw z -> z_d
                    wtz = zw.tile([P, KM_TILES, P], bf16, tag="wtz")
                    wsrc = bass.AP(tensor=w_inT.ap().tensor,
                                   offset=(DT_TILES + dt) * P,
                                   ap=[[2 * D_INNER, P], [P * 2 * D_INNER, KM_TILES], [1, P]])
                    nc.sync.dma_start(out=wtz[:], in_=wsrc)
                    for nn in range(NMM):
                        psz = zp.tile([P, 512], f32, tag="psz")
                        for k in range(KM_TILES):
                            nc.tensor.matmul(psz[:], wtz[:, k, :],
                                             ht_sb[:, k, nn * 512:(nn + 1) * 512],
                                             start=(k == 0), stop=(k == KM_TILES - 1))
                        zct = zcp.tile([P, 512], bf16, tag="zct")
                        nc.scalar.copy(out=zct[:], in_=psz[:])
                        nc.scalar.dma_start(
                            out=z_d[dt * P:(dt + 1) * P, nn * 512:(nn + 1) * 512],
                            in_=zct[:])

            # ---- boundary: one silu(z) batch + deferred chunk-0 gating ----
            # chunk-1 broadcast loads first so the DMA overlaps the batch
            for n in range(N_STATE):
                bsrc = bass.AP(tensor=xdbl_d.ap().tensor,
                               offset=(DT_RANK + n) * SEQLEN + TC,
                               ap=[[0, P], [1, TC]])
                nc.sync.dma_start(out=b_sb[n][:], in_=bsrc)
                csrc = bass.AP(tensor=xdbl_d.ap().tensor,
                               offset=(DT_RANK + N_STATE + n) * SEQLEN + TC,
                               ap=[[0, P], [1, TC]])
                nc.sync.dma_start(out=c_sb[n][:], in_=csrc)

            with tc.tile_pool(name="zf", bufs=2) as zfp:
                for dt in range(DT_TILES):
                    r0, r1 = dt * P, (dt + 1) * P
                    zf0 = zfp.tile([P, TC], bf16, tag="zf0")
                    nc.sync.dma_start(out=zf0[:], in_=z_d[r0:r1, 0:TC])
                    zf1 = zfp.tile([P, TC], bf16, tag="zf1")
                    nc.sync.dma_start(out=zf1[:], in_=z_d[r0:r1, TC:])
                    sz0 = zfp.tile([P, TC], bf16, tag="sz0")
                    nc.scalar.activation(out=sz0[:], in_=zf0[:], func=AF.Silu)
                    sz1 = zfp.tile([P, TC], bf16, tag="sz1")
                    nc.scalar.activation(out=sz1[:], in_=zf1[:], func=AF.Silu)
                    nc.scalar.dma_start(out=sz_d[r0:r1, :], in_=sz1[:])
                    pyb = zfp.tile([P, TC], bf16, tag="pyb")
                    nc.sync.dma_start(out=pyb[:], in_=py_d[r0:r1, :])
                    yf = s5y.tile([P, TC], bf16, tag="yf")
                    nc.vector.tensor_mul(out=yf[:], in0=pyb[:], in1=sz0[:])
                    nc.sync.dma_start(out=y_d[r0:r1, 0:TC], in_=yf[:])

            # ---- chunk 1 (out_proj of chunk 0 interleaved on PE) ----
            with tc.tile_pool(name="s6w", bufs=1) as s6w, \
                 tc.tile_pool(name="s6m", bufs=2) as s6m, \
                 tc.tile_pool(name="s6p", bufs=2, space="PSUM") as s6p:

                def outproj_block(c, eh, m, wo):
                    ym = s6m.tile([P, DT_TILES, P], bf16, tag="ym")
                    ysrc = bass.AP(tensor=y_d.ap().tensor,
                                   offset=(c * (TC // P) + m) * P,
                                   ap=[[SEQLEN, P], [P * SEQLEN, DT_TILES], [1, P]])
                    nc.sync.dma_start(out=ym[:], in_=ysrc)
                    ps = s6p.tile([P, 512], f32, tag="ps6")
                    for k in range(DT_TILES):
                        nc.tensor.matmul(ps[:], ym[:, k, :], wo[:, k, :],
                                         start=(k == 0), stop=(k == DT_TILES - 1))
                    ev = s6m.tile([P, 512], f32, tag="ev")
                    nc.scalar.copy(out=ev[:], in_=ps[:])
                    row = (c * (TC // P) + m) * P
                    nc.sync.dma_start(out=out[row:row + P, eh * 512:(eh + 1) * 512],
                                      in_=ev[:])

                def load_wo(eh):
                    wo = s6w.tile([P, DT_TILES, 512], bf16, tag="wo")
                    wsrc = bass.AP(tensor=w_outT.ap().tensor, offset=eh * 512,
                                   ap=[[D_MODEL, P], [P * D_MODEL, DT_TILES], [1, 512]])
                    nc.sync.dma_start(out=wo[:], in_=wsrc)
                    return wo

                wo = load_wo(0)
                for dt in range(DT_TILES):
                    py, _ = scan_tile(1, dt)
                    szt = s5s.tile([P, TC], bf16, tag="szt")
                    nc.sync.dma_start(out=szt[:], in_=sz_d[dt * P:(dt + 1) * P, :])
                    yf = s5y.tile([P, TC], bf16, tag="yf")
                    nc.vector.tensor_mul(out=yf[:], in0=py[:], in1=szt[:])
                    nc.sync.dma_start(out=y_d[dt * P:(dt + 1) * P, TC:], in_=yf[:])
                    # interleave chunk-0 out_proj (8 blocks per eh half)
                    if dt == 8:
                        wo = load_wo(1)
                    outproj_block(0, dt // 8, dt % 8, wo)

                # chunk-1 out_proj tail
                for eh in range(2):
                    wo = load_wo(eh)
                    for m in range(TC // P):
                        outproj_block(1, eh, m, wo)
        stack.close()

    nc.compile()
    return nc


def kernel(hidden_states, in_proj_w, conv_w_f, conv_b_f, conv_w_b, conv_b_b,
           x_proj_w_f, dt_proj_w_f, dt_proj_b_f, x_proj_w_b, dt_proj_w_b, dt_proj_b_b,
           A_log_f, A_log_b, D_f, D_b, out_proj_w):
    from concourse.bass_utils import run_bass_kernel_spmd

    if "nc" not in _CACHE:
        _CACHE["nc"] = _build()
    nc = _CACHE["nc"]

    f = np.ascontiguousarray
    import ml_dtypes
    w_inT = f(np.asarray(in_proj_w).T.astype(ml_dtypes.bfloat16))
    w_outT = f((np.asarray(out_proj_w).T.astype(np.float32) * 0.5).astype(ml_dtypes.bfloat16))
    per_dir = {}
    for d, (cw, cb, xp, dtp, dtb, alog, dv) in {
        0: (conv_w_f, conv_b_f, x_proj_w_f, dt_proj_w_f, dt_proj_b_f, A_log_f, D_f),
        1: (conv_w_b, conv_b_b, x_proj_w_b, dt_proj_w_b, dt_proj_b_b, A_log_b, D_b),
    }.items():
        per_dir[d] = {
            "conv_w": f(np.asarray(cw).reshape(D_INNER, K_CONV).astype(np.float32)),
            "conv_b": f(np.asarray(cb).reshape(D_INNER, 1).astype(np.float32)),
            "x_projT": f(np.asarray(xp).T.astype(ml_dtypes.bfloat16)),
            "dt_projT": f(np.asarray(dtp).T.astype(ml_dtypes.bfloat16)),
            "dt_b": f(np.asarray(dtb).reshape(D_INNER, 1).astype(np.float32)),
            "A_m": f((-np.exp(np.asarray(alog))).astype(np.float32)),
            "D_v": f(np.asarray(dv).reshape(D_INNER, 1).astype(np.float32)),
        }

    hidden_states = np.asarray(hidden_states)
    in_maps = []
    for c in range(8):
        b, d = c % BATCH, c // BATCH
        h = hidden_states[b].T if d == 0 else hidden_states[b][::-1].T
        m = {"hT": f(h.astype(ml_dtypes.bfloat16)), "w_inT": w_inT, "w_outT": w_outT}
        m.update(per_dir[d])
        in_maps.append(m)

    _CACHE["in_maps"] = in_maps
    global _LAST_IN_MAPS
    _LAST_IN_MAPS = in_maps
    res = run_bass_kernel_spmd(nc, in_maps, list(range(8)))
    outs = [res.results[i]["out"] for i in range(8)]
    result = np.empty((BATCH, SEQLEN, D_MODEL), np.float32)
    for b in range(BATCH):
        result[b] = outs[b] + outs[BATCH + b][::-1, :]
    return result
